# revision 48
# baseline (speedup 1.0000x reference)
"""Trainium2 Bass kernel for nn_Aggregator (segment_reduce).

Math: the reference's gather+einsum collapses algebraically:
  out[b, o] = sum_p sum_k x[b, p, indices[k]] * Wx[o, p] + const[o]
            = sum_p (sum_v count[v] * x[b, p, v]) * Wx[o, p] + const[o]
where count = histogram of `indices` and const = index-path output + K*b_aggre
(both batch-independent, computed on host from the tiny weights).

Device pipeline per core (B_loc = 2048 rows, batch on partitions, 16 tiles
of 128 rows, fully per-tile pipelined with deep DMA prefetch):
  1. DMA x tile [128, 4704]
  2. DVE: weighted view-reduction, one fused scalar_tensor_tensor per
     distinct view, split into q-halves so the strided read walks 192 B
     (2x faster per element than the natural 96 B stride)
  3. PE: transpose x_summed [128, 197] -> [197, 128] chunks (ones column
     folds the const row into the stage-2 contraction)
  4. PE: fp32 matmul [p, b]^T @ WxT[p, o] accumulating in PSUM
  5. ACT: PSUM -> SBUF copies; DMA out [128, 1024] tile

Sharding: batch dim split 8 ways (2048 rows/core); small weights replicated.
"""

import numpy as np

import concourse.bass as bass
import concourse.mybir as mybir
from concourse import bacc
from concourse.tile import TileContext
from concourse.bass_utils import run_bass_kernel_spmd

N_CORES = 8
B_TOTAL = 16384
B_LOC = B_TOTAL // N_CORES  # 2048
P = 196  # 14*14 spatial features
HALF_Q = 98  # q-split: stage-1 APs walk q in two halves (stride 192 B)
V = 24  # views
F = P * V  # 4704 features per batch row
OUT = 1024
TILE = 128
N_TILES = B_LOC // TILE  # 16
N_PRE = 1  # x loads issued ahead of the weight loads
C1 = HALF_Q  # first contraction chunk: acc cols 0..97 (q-split half 0)
C2 = HALF_Q + 1  # second chunk: cols 98..195 plus the ones column (99 rows)
PW = 512  # PSUM half width (max fp32 moving cols)

_cache = {}
last_results = None  # BassKernelResults of the most recent run (for test harness)


def _build_program(count_items):
    """count_items: tuple of (view, count) with count > 0."""
    nc = bacc.Bacc(
        "TRN2",
        target_bir_lowering=False,
        debug=False,
        num_devices=N_CORES,
    )
    f32 = mybir.dt.float32
    x_in = nc.declare_dram_parameter("x_in", [B_LOC, F], f32, isOutput=False)
    wxa = nc.declare_dram_parameter("wxa", [C1, OUT], f32, isOutput=False)
    wxb = nc.declare_dram_parameter("wxb", [C2, OUT], f32, isOutput=False)
    ident = nc.declare_dram_parameter("ident", [128, 128], f32, isOutput=False)
    out = nc.declare_dram_parameter("out", [B_LOC, OUT], f32, isOutput=True)

    MU = mybir.AluOpType.mult
    AD = mybir.AluOpType.add

    with TileContext(nc) as tc:
        with (
            tc.tile_pool(name="consts", bufs=1) as cpool,
            tc.tile_pool(name="x", bufs=4) as xpool,
            tc.tile_pool(name="acc", bufs=3) as apool,
            tc.tile_pool(name="lhs", bufs=6) as lpool,
            tc.tile_pool(name="outs", bufs=3) as opool,
            tc.tile_pool(name="pt", bufs=2, space="PSUM") as ptpool,
            tc.tile_pool(name="po", bufs=2, space="PSUM") as popool,
        ):
            # group schedule: (start_tile, n_tiles); small edge groups for a
            # fast pipeline ramp-in and a short drain tail
            sizes = [1, 2, 2, 2, 2, 2, 2, 2, 1]
            assert sum(sizes) == N_TILES
            groups = []
            t0 = 0
            for sz in sizes:
                groups.append((t0, sz))
                t0 += sz
            N_GRP = len(groups)
            xtiles = [None] * N_GRP

            # issue the first x loads before the weight loads so the critical
            # first tile isn't queued behind them
            def load(g):
                t0, sz = groups[g]
                xt = xpool.tile([TILE, 2, F], f32, tag="xt")
                nc.sync.dma_start(
                    out=xt[:, 0:sz, :],
                    in_=x_in[t0 * TILE : (t0 + sz) * TILE, :].rearrange(
                        "(g p) f -> p g f", p=TILE
                    ),
                )
                xtiles[g] = xt

            for g in range(N_PRE):
                load(g)

            wxa_t = cpool.tile([C1, OUT], f32)
            nc.sync.dma_start(out=wxa_t[:], in_=wxa[:])
            wxb_t = cpool.tile([C2, OUT], f32)
            nc.sync.dma_start(out=wxb_t[:], in_=wxb[:])
            id_t = cpool.tile([128, 128], f32)
            nc.sync.dma_start(out=id_t[:], in_=ident[:])

            def stage1(g):
                """Weighted view-reduction for one group of tiles."""
                if xtiles[g] is None:
                    load(g)
                xt = xtiles[g]
                sz = groups[g][1]
                acc = apool.tile([TILE, 2, P + 1], f32, tag="acc")
                # ones column: folds const into the stage-2 contraction
                nc.vector.memset(acc[:, 0:sz, P : P + 1], 1.0)
                # stage-1 views with q split in halves: x walks 192 B strides
                xh = xt.rearrange("p g (q h v) -> p g q h v", v=V, h=2)
                ah = acc[:, :, 0:P].rearrange("p g (h q) -> p g h q", h=2)
                for h in range(2):
                    for i, (v, c) in enumerate(count_items):
                        xs = xh[:, 0:sz, :, h, v]
                        dst = ah[:, 0:sz, h, :]
                        if i == 0:
                            nc.vector.tensor_scalar_mul(dst, xs, float(c))
                        else:
                            nc.vector.scalar_tensor_tensor(
                                out=dst, in0=xs, scalar=float(c), in1=dst,
                                op0=MU, op1=AD,
                            )
                return acc

            def stage2(t, acc, j):
                g_in = acc[:, j, :]
                pt1 = ptpool.tile([C1, 128], f32, tag="pt1")
                nc.tensor.transpose(pt1[:], g_in[:, 0:C1], id_t[:])
                pt2 = ptpool.tile([C2, 128], f32, tag="pt2")
                nc.tensor.transpose(pt2[:], g_in[:, C1 : P + 1], id_t[:])
                l1 = lpool.tile([C1, 128], f32, tag="l1")
                nc.scalar.copy(l1[:], pt1[:])
                l2 = lpool.tile([C2, 128], f32, tag="l2")
                nc.scalar.copy(l2[:], pt2[:])
                ot = opool.tile([TILE, OUT], f32, tag="ot")
                po1 = popool.tile([128, PW], f32, tag="po1")
                nc.tensor.matmul(po1[:], l1[:], wxa_t[:, 0:PW], start=True, stop=False)
                nc.tensor.matmul(po1[:], l2[:], wxb_t[:, 0:PW], start=False, stop=True)
                nc.scalar.copy(ot[:, 0:PW], po1[:])
                po2 = popool.tile([128, PW], f32, tag="po2")
                nc.tensor.matmul(po2[:], l1[:], wxa_t[:, PW:OUT], start=True, stop=False)
                nc.tensor.matmul(po2[:], l2[:], wxb_t[:, PW:OUT], start=False, stop=True)
                nc.scalar.copy(ot[:, PW:OUT], po2[:])
                nc.sync.dma_start(out=out[t * TILE : (t + 1) * TILE, :], in_=ot[:])

            # burst phasing: 2 groups of stage-1, then their stage-2 blocks,
            # so PE work comes in dense bursts (fewer HAM cold ramps)
            pairs = [
                [g for g in (2 * bp, 2 * bp + 1) if g < N_GRP]
                for bp in range((N_GRP + 1) // 2)
            ]
            for pair in pairs:
                accs = [stage1(g) for g in pair]
                for g, a in zip(pair, accs):
                    for j in range(groups[g][1]):
                        stage2(groups[g][0] + j, a, j)
    nc.finalize()
    return nc


def kernel(x, indices, W_idx, b_idx, W_aggre, b_aggre, **run_kwargs):
    global last_results
    x = np.ascontiguousarray(np.asarray(x, dtype=np.float32)).reshape(B_TOTAL, F)
    idx = np.asarray(indices).astype(np.int64).ravel()
    W_idx = np.asarray(W_idx, dtype=np.float32)
    b_idx = np.asarray(b_idx, dtype=np.float32)
    W_aggre = np.asarray(W_aggre, dtype=np.float32)
    b_aggre = np.asarray(b_aggre, dtype=np.float32)

    count = np.bincount(idx, minlength=V)
    count_items = tuple((int(v), float(count[v])) for v in range(V) if count[v] != 0)

    # batch-independent index path, on host (tiny: [24, 196] scale)
    z = W_idx[:, idx].T + b_idx  # [K, 196]
    feats = np.where(z > 0, z, np.float32(0.2) * z).astype(np.float32)
    Wi = W_aggre[:, P:]
    const = feats.sum(0, dtype=np.float32) @ Wi.T + np.float32(len(idx)) * b_aggre

    WxT = np.ascontiguousarray(W_aggre[:, :P].T)  # [196, 1024]
    # stage-1 writes x_summed[p] to acc column j = (p%2)*98 + p//2 (q-split
    # interleave); permute WxT rows to match so the contraction stays aligned
    p_of_j = np.array(
        [2 * (j % HALF_Q) + j // HALF_Q for j in range(P)], dtype=np.int64
    )
    WxTp = WxT[p_of_j]
    wxa_np = np.ascontiguousarray(WxTp[:C1])
    wxb_np = np.ascontiguousarray(
        np.concatenate([WxTp[C1:], const[None, :].astype(np.float32)], axis=0)
    )
    ident_np = np.eye(128, dtype=np.float32)

    nc = _cache.get(count_items)
    if nc is None:
        nc = _build_program(count_items)
        _cache[count_items] = nc

    in_maps = [
        {
            "x_in": np.ascontiguousarray(x[i * B_LOC : (i + 1) * B_LOC]),
            "wxa": wxa_np,
            "wxb": wxb_np,
            "ident": ident_np,
        }
        for i in range(N_CORES)
    ]
    res = run_bass_kernel_spmd(nc, in_maps, core_ids=list(range(N_CORES)), **run_kwargs)
    last_results = res
    return np.concatenate([res.results[i]["out"] for i in range(N_CORES)], axis=0)


# revision 49
# speedup vs baseline: 1.0022x; 1.0022x over previous
"""Trainium2 Bass kernel for nn_Aggregator (segment_reduce).

Math: the reference's gather+einsum collapses algebraically:
  out[b, o] = sum_p sum_k x[b, p, indices[k]] * Wx[o, p] + const[o]
            = sum_p (sum_v count[v] * x[b, p, v]) * Wx[o, p] + const[o]
where count = histogram of `indices` and const = index-path output + K*b_aggre
(both batch-independent, computed on host from the tiny weights).

Device pipeline per core (B_loc = 2048 rows, batch on partitions, 16 tiles
of 128 rows, fully per-tile pipelined with deep DMA prefetch):
  1. DMA x tile [128, 4704]
  2. DVE: weighted view-reduction, one fused scalar_tensor_tensor per
     distinct view, split into q-halves so the strided read walks 192 B
     (2x faster per element than the natural 96 B stride)
  3. PE: transpose x_summed [128, 197] -> [197, 128] chunks (ones column
     folds the const row into the stage-2 contraction)
  4. PE: fp32 matmul [p, b]^T @ WxT[p, o] accumulating in PSUM
  5. ACT: PSUM -> SBUF copies; DMA out [128, 1024] tile

Sharding: batch dim split 8 ways (2048 rows/core); small weights replicated.
"""

import numpy as np

import concourse.bass as bass
import concourse.mybir as mybir
from concourse import bacc
from concourse.tile import TileContext
from concourse.bass_utils import run_bass_kernel_spmd

N_CORES = 8
B_TOTAL = 16384
B_LOC = B_TOTAL // N_CORES  # 2048
P = 196  # 14*14 spatial features
HALF_Q = 98  # q-split: stage-1 APs walk q in two halves (stride 192 B)
V = 24  # views
F = P * V  # 4704 features per batch row
OUT = 1024
TILE = 128
N_TILES = B_LOC // TILE  # 16
N_PRE = 1  # x loads issued ahead of the weight loads
C1 = HALF_Q  # first contraction chunk: acc cols 0..97 (q-split half 0)
C2 = HALF_Q + 1  # second chunk: cols 98..195 plus the ones column (99 rows)
PW = 512  # PSUM half width (max fp32 moving cols)

_cache = {}
last_results = None  # BassKernelResults of the most recent run (for test harness)


def _build_program(count_items):
    """count_items: tuple of (view, count) with count > 0."""
    nc = bacc.Bacc(
        "TRN2",
        target_bir_lowering=False,
        debug=False,
        num_devices=N_CORES,
    )
    f32 = mybir.dt.float32
    x_in = nc.declare_dram_parameter("x_in", [B_LOC, F], f32, isOutput=False)
    wxa = nc.declare_dram_parameter("wxa", [C1, OUT], f32, isOutput=False)
    wxb = nc.declare_dram_parameter("wxb", [C2, OUT], f32, isOutput=False)
    ident = nc.declare_dram_parameter("ident", [128, 128], f32, isOutput=False)
    out = nc.declare_dram_parameter("out", [B_LOC, OUT], f32, isOutput=True)

    MU = mybir.AluOpType.mult
    AD = mybir.AluOpType.add

    with TileContext(nc) as tc:
        with (
            tc.tile_pool(name="consts", bufs=1) as cpool,
            tc.tile_pool(name="x", bufs=4) as xpool,
            tc.tile_pool(name="acc", bufs=3) as apool,
            tc.tile_pool(name="lhs", bufs=6) as lpool,
            tc.tile_pool(name="outs", bufs=3) as opool,
            tc.tile_pool(name="pt", bufs=2, space="PSUM") as ptpool,
            tc.tile_pool(name="po", bufs=2, space="PSUM") as popool,
        ):
            # group schedule: (start_tile, n_tiles); small edge groups for a
            # fast pipeline ramp-in and a short drain tail
            sizes = [2] * 8
            assert sum(sizes) == N_TILES
            groups = []
            t0 = 0
            for sz in sizes:
                groups.append((t0, sz))
                t0 += sz
            N_GRP = len(groups)
            xtiles = [None] * N_GRP

            # issue the first x loads before the weight loads so the critical
            # first tile isn't queued behind them
            def load(g):
                t0, sz = groups[g]
                xt = xpool.tile([TILE, 2, F], f32, tag="xt")
                nc.sync.dma_start(
                    out=xt[:, 0:sz, :],
                    in_=x_in[t0 * TILE : (t0 + sz) * TILE, :].rearrange(
                        "(g p) f -> p g f", p=TILE
                    ),
                )
                xtiles[g] = xt

            for g in range(N_PRE):
                load(g)

            wxa_t = cpool.tile([C1, OUT], f32)
            nc.sync.dma_start(out=wxa_t[:], in_=wxa[:])
            wxb_t = cpool.tile([C2, OUT], f32)
            nc.sync.dma_start(out=wxb_t[:], in_=wxb[:])
            id_t = cpool.tile([128, 128], f32)
            nc.sync.dma_start(out=id_t[:], in_=ident[:])

            def stage1(g):
                """Weighted view-reduction for one group of tiles."""
                if xtiles[g] is None:
                    load(g)
                xt = xtiles[g]
                sz = groups[g][1]
                acc = apool.tile([TILE, 2, P + 1], f32, tag="acc")
                # ones column: folds const into the stage-2 contraction
                nc.vector.memset(acc[:, 0:sz, P : P + 1], 1.0)
                # stage-1 views with q split in halves: x walks 192 B strides
                xh = xt.rearrange("p g (q h v) -> p g q h v", v=V, h=2)
                ah = acc[:, :, 0:P].rearrange("p g (h q) -> p g h q", h=2)
                for h in range(2):
                    for i, (v, c) in enumerate(count_items):
                        xs = xh[:, 0:sz, :, h, v]
                        dst = ah[:, 0:sz, h, :]
                        if i == 0:
                            nc.vector.tensor_scalar_mul(dst, xs, float(c))
                        else:
                            nc.vector.scalar_tensor_tensor(
                                out=dst, in0=xs, scalar=float(c), in1=dst,
                                op0=MU, op1=AD,
                            )
                return acc

            def stage2(t, acc, j):
                g_in = acc[:, j, :]
                pt1 = ptpool.tile([C1, 128], f32, tag="pt1")
                nc.tensor.transpose(pt1[:], g_in[:, 0:C1], id_t[:])
                pt2 = ptpool.tile([C2, 128], f32, tag="pt2")
                nc.tensor.transpose(pt2[:], g_in[:, C1 : P + 1], id_t[:])
                l1 = lpool.tile([C1, 128], f32, tag="l1")
                nc.scalar.copy(l1[:], pt1[:])
                l2 = lpool.tile([C2, 128], f32, tag="l2")
                nc.scalar.copy(l2[:], pt2[:])
                ot = opool.tile([TILE, OUT], f32, tag="ot")
                po1 = popool.tile([128, PW], f32, tag="po1")
                nc.tensor.matmul(po1[:], l1[:], wxa_t[:, 0:PW], start=True, stop=False)
                nc.tensor.matmul(po1[:], l2[:], wxb_t[:, 0:PW], start=False, stop=True)
                nc.scalar.copy(ot[:, 0:PW], po1[:])
                po2 = popool.tile([128, PW], f32, tag="po2")
                nc.tensor.matmul(po2[:], l1[:], wxa_t[:, PW:OUT], start=True, stop=False)
                nc.tensor.matmul(po2[:], l2[:], wxb_t[:, PW:OUT], start=False, stop=True)
                nc.scalar.copy(ot[:, PW:OUT], po2[:])
                nc.sync.dma_start(out=out[t * TILE : (t + 1) * TILE, :], in_=ot[:])

            # burst phasing: 2 groups of stage-1, then their stage-2 blocks,
            # so PE work comes in dense bursts (fewer HAM cold ramps)
            pairs = [
                [g for g in (2 * bp, 2 * bp + 1) if g < N_GRP]
                for bp in range((N_GRP + 1) // 2)
            ]
            for pair in pairs:
                accs = [stage1(g) for g in pair]
                for g, a in zip(pair, accs):
                    for j in range(groups[g][1]):
                        stage2(groups[g][0] + j, a, j)
    nc.finalize()
    return nc


def kernel(x, indices, W_idx, b_idx, W_aggre, b_aggre, **run_kwargs):
    global last_results
    x = np.ascontiguousarray(np.asarray(x, dtype=np.float32)).reshape(B_TOTAL, F)
    idx = np.asarray(indices).astype(np.int64).ravel()
    W_idx = np.asarray(W_idx, dtype=np.float32)
    b_idx = np.asarray(b_idx, dtype=np.float32)
    W_aggre = np.asarray(W_aggre, dtype=np.float32)
    b_aggre = np.asarray(b_aggre, dtype=np.float32)

    count = np.bincount(idx, minlength=V)
    count_items = tuple((int(v), float(count[v])) for v in range(V) if count[v] != 0)

    # batch-independent index path, on host (tiny: [24, 196] scale)
    z = W_idx[:, idx].T + b_idx  # [K, 196]
    feats = np.where(z > 0, z, np.float32(0.2) * z).astype(np.float32)
    Wi = W_aggre[:, P:]
    const = feats.sum(0, dtype=np.float32) @ Wi.T + np.float32(len(idx)) * b_aggre

    WxT = np.ascontiguousarray(W_aggre[:, :P].T)  # [196, 1024]
    # stage-1 writes x_summed[p] to acc column j = (p%2)*98 + p//2 (q-split
    # interleave); permute WxT rows to match so the contraction stays aligned
    p_of_j = np.array(
        [2 * (j % HALF_Q) + j // HALF_Q for j in range(P)], dtype=np.int64
    )
    WxTp = WxT[p_of_j]
    wxa_np = np.ascontiguousarray(WxTp[:C1])
    wxb_np = np.ascontiguousarray(
        np.concatenate([WxTp[C1:], const[None, :].astype(np.float32)], axis=0)
    )
    ident_np = np.eye(128, dtype=np.float32)

    nc = _cache.get(count_items)
    if nc is None:
        nc = _build_program(count_items)
        _cache[count_items] = nc

    in_maps = [
        {
            "x_in": np.ascontiguousarray(x[i * B_LOC : (i + 1) * B_LOC]),
            "wxa": wxa_np,
            "wxb": wxb_np,
            "ident": ident_np,
        }
        for i in range(N_CORES)
    ]
    res = run_bass_kernel_spmd(nc, in_maps, core_ids=list(range(N_CORES)), **run_kwargs)
    last_results = res
    return np.concatenate([res.results[i]["out"] for i in range(N_CORES)], axis=0)


# revision 50
# speedup vs baseline: 1.2112x; 1.2085x over previous
"""Trainium2 Bass kernel for nn_Aggregator (segment_reduce).

Math: the reference's gather+einsum collapses algebraically:
  out[b, o] = sum_p sum_k x[b, p, indices[k]] * Wx[o, p] + const[o]
            = sum_p (sum_v count[v] * x[b, p, v]) * Wx[o, p] + const[o]
where count = histogram of `indices` and const = index-path output + K*b_aggre
(both batch-independent, computed on host from the tiny weights).

Device pipeline per core (B_loc = 2048 rows, batch on partitions, 16 tiles
of 128 rows, fully per-tile pipelined with deep DMA prefetch):
  1. DMA x tile [128, 4704]
  2. DVE: weighted view-reduction, one fused scalar_tensor_tensor per
     distinct view, split into q-halves so the strided read walks 192 B
     (2x faster per element than the natural 96 B stride)
  3. PE: transpose x_summed [128, 197] -> [197, 128] chunks (ones column
     folds the const row into the stage-2 contraction)
  4. PE: fp32 matmul [p, b]^T @ WxT[p, o] accumulating in PSUM
  5. ACT: PSUM -> SBUF copies; DMA out [128, 1024] tile

Sharding: batch dim split 8 ways (2048 rows/core); small weights replicated.
"""

import numpy as np

import concourse.bass as bass
import concourse.mybir as mybir
from concourse import bacc
from concourse.tile import TileContext
from concourse.bass_utils import run_bass_kernel_spmd

N_CORES = 8
B_TOTAL = 16384
B_LOC = B_TOTAL // N_CORES  # 2048
P = 196  # 14*14 spatial features
HALF_Q = 98  # q-split: stage-1 APs walk q in two halves (stride 192 B)
V = 24  # views
F = P * V  # 4704 features per batch row
OUT = 1024
TILE = 128
N_TILES = B_LOC // TILE  # 16
N_PRE = 1  # x loads issued ahead of the weight loads
C1 = HALF_Q  # first contraction chunk: acc cols 0..97 (q-split half 0)
C2 = HALF_Q + 1  # second chunk: cols 98..195 plus the ones column (99 rows)
PW = 512  # PSUM half width (max fp32 moving cols)

_cache = {}
last_results = None  # BassKernelResults of the most recent run (for test harness)


def _build_program(count_items):
    """count_items: tuple of (view, count) with count > 0."""
    nc = bacc.Bacc(
        "TRN2",
        target_bir_lowering=False,
        debug=False,
        num_devices=N_CORES,
    )
    f32 = mybir.dt.float32
    x_in = nc.declare_dram_parameter("x_in", [B_LOC, F], f32, isOutput=False)
    wxa = nc.declare_dram_parameter("wxa", [C1, OUT], f32, isOutput=False)
    wxb = nc.declare_dram_parameter("wxb", [C2, OUT], f32, isOutput=False)
    ident = nc.declare_dram_parameter("ident", [128, 128], f32, isOutput=False)
    out = nc.declare_dram_parameter("out", [B_LOC, OUT], f32, isOutput=True)

    MU = mybir.AluOpType.mult
    AD = mybir.AluOpType.add

    with TileContext(nc) as tc:
        with (
            tc.tile_pool(name="consts", bufs=1) as cpool,
            tc.tile_pool(name="x", bufs=4) as xpool,
            tc.tile_pool(name="acc", bufs=3) as apool,
            tc.tile_pool(name="lhs", bufs=6) as lpool,
            tc.tile_pool(name="outs", bufs=3) as opool,
            tc.tile_pool(name="pt", bufs=2, space="PSUM") as ptpool,
            tc.tile_pool(name="po", bufs=2, space="PSUM") as popool,
        ):
            # group schedule: (start_tile, n_tiles); small edge groups for a
            # fast pipeline ramp-in and a short drain tail
            sizes = [2] * 8
            assert sum(sizes) == N_TILES
            groups = []
            t0 = 0
            for sz in sizes:
                groups.append((t0, sz))
                t0 += sz
            N_GRP = len(groups)
            xtiles = [None] * N_GRP

            # issue the first x loads before the weight loads so the critical
            # first tile isn't queued behind them
            def load(g):
                t0, sz = groups[g]
                xt = xpool.tile([TILE, 2, F], f32, tag="xt")
                nc.sync.dma_start(
                    out=xt[:, 0:sz, :],
                    in_=x_in[t0 * TILE : (t0 + sz) * TILE, :].rearrange(
                        "(g p) f -> p g f", p=TILE
                    ),
                )
                xtiles[g] = xt

            for g in range(N_PRE):
                load(g)

            wxa_t = cpool.tile([C1, OUT], f32)
            nc.sync.dma_start(out=wxa_t[:], in_=wxa[:])
            wxb_t = cpool.tile([C2, OUT], f32)
            nc.sync.dma_start(out=wxb_t[:], in_=wxb[:])
            id_t = cpool.tile([128, 128], f32)
            nc.sync.dma_start(out=id_t[:], in_=ident[:])

            def stage1(g):
                """Weighted view-reduction for one group of tiles."""
                if xtiles[g] is None:
                    load(g)
                xt = xtiles[g]
                sz = groups[g][1]
                acc = apool.tile([TILE, 2, P + 1], f32, tag="acc")
                # ones column: folds const into the stage-2 contraction
                nc.vector.memset(acc[:, 0:sz, P : P + 1], 1.0)
                # stage-1 views with q split in halves: x walks 192 B strides
                xh = xt.rearrange("p g (q h v) -> p g q h v", v=V, h=2)
                ah = acc[:, :, 0:P].rearrange("p g (h q) -> p g h q", h=2)
                for h in range(2):
                    for i, (v, c) in enumerate(count_items):
                        xs = xh[:, 0:sz, :, h, v]
                        dst = ah[:, 0:sz, h, :]
                        if i == 0:
                            nc.vector.tensor_scalar_mul(dst, xs, float(c))
                        else:
                            nc.vector.scalar_tensor_tensor(
                                out=dst, in0=xs, scalar=float(c), in1=dst,
                                op0=MU, op1=AD,
                            )
                return acc

            def stage2(t, acc, j):
                g_in = acc[:, j, :]
                pt1 = ptpool.tile([C1, 128], f32, tag="pt1")
                nc.tensor.transpose(pt1[:], g_in[:, 0:C1], id_t[:])
                pt2 = ptpool.tile([C2, 128], f32, tag="pt2")
                nc.tensor.transpose(pt2[:], g_in[:, C1 : P + 1], id_t[:])
                l1 = lpool.tile([C1, 128], f32, tag="l1")
                nc.scalar.copy(l1[:], pt1[:])
                l2 = lpool.tile([C2, 128], f32, tag="l2")
                nc.scalar.copy(l2[:], pt2[:])
                ot = opool.tile([TILE, OUT], f32, tag="ot")
                po1 = popool.tile([128, PW], f32, tag="po1")
                nc.tensor.matmul(po1[:], l1[:], wxa_t[:, 0:PW], start=True, stop=False)
                nc.tensor.matmul(po1[:], l2[:], wxb_t[:, 0:PW], start=False, stop=True)
                nc.scalar.copy(ot[:, 0:PW], po1[:])
                po2 = popool.tile([128, PW], f32, tag="po2")
                nc.tensor.matmul(po2[:], l1[:], wxa_t[:, PW:OUT], start=True, stop=False)
                nc.tensor.matmul(po2[:], l2[:], wxb_t[:, PW:OUT], start=False, stop=True)
                nc.scalar.copy(ot[:, PW:OUT], po2[:])
                # stores ride the ACT HWDGE ring, loads the SP ring
                nc.scalar.dma_start(out=out[t * TILE : (t + 1) * TILE, :], in_=ot[:])

            # burst phasing: 2 groups of stage-1, then their stage-2 blocks,
            # so PE work comes in dense bursts (fewer HAM cold ramps)
            pairs = [
                [g for g in (2 * bp, 2 * bp + 1) if g < N_GRP]
                for bp in range((N_GRP + 1) // 2)
            ]
            for pair in pairs:
                accs = [stage1(g) for g in pair]
                for g, a in zip(pair, accs):
                    for j in range(groups[g][1]):
                        stage2(groups[g][0] + j, a, j)
    nc.finalize()
    return nc


def kernel(x, indices, W_idx, b_idx, W_aggre, b_aggre, **run_kwargs):
    global last_results
    x = np.ascontiguousarray(np.asarray(x, dtype=np.float32)).reshape(B_TOTAL, F)
    idx = np.asarray(indices).astype(np.int64).ravel()
    W_idx = np.asarray(W_idx, dtype=np.float32)
    b_idx = np.asarray(b_idx, dtype=np.float32)
    W_aggre = np.asarray(W_aggre, dtype=np.float32)
    b_aggre = np.asarray(b_aggre, dtype=np.float32)

    count = np.bincount(idx, minlength=V)
    count_items = tuple((int(v), float(count[v])) for v in range(V) if count[v] != 0)

    # batch-independent index path, on host (tiny: [24, 196] scale)
    z = W_idx[:, idx].T + b_idx  # [K, 196]
    feats = np.where(z > 0, z, np.float32(0.2) * z).astype(np.float32)
    Wi = W_aggre[:, P:]
    const = feats.sum(0, dtype=np.float32) @ Wi.T + np.float32(len(idx)) * b_aggre

    WxT = np.ascontiguousarray(W_aggre[:, :P].T)  # [196, 1024]
    # stage-1 writes x_summed[p] to acc column j = (p%2)*98 + p//2 (q-split
    # interleave); permute WxT rows to match so the contraction stays aligned
    p_of_j = np.array(
        [2 * (j % HALF_Q) + j // HALF_Q for j in range(P)], dtype=np.int64
    )
    WxTp = WxT[p_of_j]
    wxa_np = np.ascontiguousarray(WxTp[:C1])
    wxb_np = np.ascontiguousarray(
        np.concatenate([WxTp[C1:], const[None, :].astype(np.float32)], axis=0)
    )
    ident_np = np.eye(128, dtype=np.float32)

    nc = _cache.get(count_items)
    if nc is None:
        nc = _build_program(count_items)
        _cache[count_items] = nc

    in_maps = [
        {
            "x_in": np.ascontiguousarray(x[i * B_LOC : (i + 1) * B_LOC]),
            "wxa": wxa_np,
            "wxb": wxb_np,
            "ident": ident_np,
        }
        for i in range(N_CORES)
    ]
    res = run_bass_kernel_spmd(nc, in_maps, core_ids=list(range(N_CORES)), **run_kwargs)
    last_results = res
    return np.concatenate([res.results[i]["out"] for i in range(N_CORES)], axis=0)
